# revision 6
# baseline (speedup 1.0000x reference)
"""Bass/Trainium2 kernel for nn_Attention_369367188096 (sparse_attention).

Reference computation (B=2, N=4096, IN_DIM=1024, DIM=1024, HEADS=8, d=128):
    qkv = x @ W_qkv ; split into q,k,v per head
    dots = (q @ k^T) * DIM**-0.5 ; masked on top-left [2048,2048] block
    attn = softmax(dots) ; out = attn @ v ; out @ W_out + b_out

Sharding across 8 NeuronCores: core i handles batch b=i//4 and heads
(2*(i%4), 2*(i%4)+1).  Each core computes a partial output
x[b]-rows x DIM using its two heads' slice of W_out (row-sharded);
the host sums 4 partials per batch and adds b_out.

All matmul operands are bf16 (PE runs bf16 at 1 cycle/row vs 4x for fp32);
accumulation is fp32 in PSUM.  Softmax uses no max-subtraction: scores are
|s| <~ 1.5 after the 1/32 scale, so exp is numerically safe, and masking is
an exact 0/1 multiply after exp (identical to exp(-inf)=0).

Device dataflow (all layouts chosen so matmuls only ever stream, never
transpose): Q^T,K^T = W.T @ x^T with W chunks as PE weights; V natural via
x^T chunks as weights; S^T = K Q^T per (j-chunk, i-group of 512); exp on
ScalarE (scale folded in), 0/1 mask multiply on VectorE; out^T accumulates
V.T @ exp(S^T); the softmax denominator rides a ones-weights matmul whose
output is already broadcast across partitions; 1/den = exp(-ln(den)) on
ScalarE (both functions live in one activation table set); out^T slices are
exactly the lhsT the output projection needs.
"""

import os
import sys

for _p in ("/opt/trn_rl_repo", "/root/.axon_site/_ro/trn_rl_repo"):
    if os.path.isdir(_p) and _p not in sys.path:
        sys.path.insert(0, _p)

from contextlib import ExitStack

import ml_dtypes
import numpy as np

import concourse.bass as bass
import concourse.bacc as bacc
import concourse.mybir as mybir
import concourse.tile as tile
from concourse.bass_utils import run_bass_kernel_spmd

BF16 = mybir.dt.bfloat16
F32 = mybir.dt.float32
P = 128          # partitions
IN_DIM = 1024    # model in dim
OUT_DIM = 1024   # model out dim
DH = 128         # head dim
NH = 2           # heads per core
FD = 512         # matmul moving free dim
N_FULL = 4096    # sequence length
MM_FULL = 2048   # masked block size
SCALE = 1024 ** -0.5
N_CORES = 8


def build_nc(n=N_FULL, mm=MM_FULL):
    """Build the per-core Bass program (SPMD: same program, per-core data)."""
    CI = IN_DIM // P          # 8 input-dim chunks
    JC = n // P               # key chunks (32)
    IG = n // FD              # query groups of 512 (8)
    MJ = mm // P              # masked key chunks (16)
    MG = mm // FD             # masked query groups (4)
    assert MJ % 2 == 0 and JC % 2 == 0
    AF = mybir.ActivationFunctionType

    nc = bacc.Bacc("TRN2", target_bir_lowering=False, debug=False)
    # W tensors arrive host-prelayouted with 128 partitions contiguous so the
    # DMAs are dense and fast (they gate the first matmul).
    wq_d = nc.dram_tensor("wq", [P, CI * NH * DH], BF16, kind="ExternalInput")
    wk_d = nc.dram_tensor("wk", [P, CI * NH * DH], BF16, kind="ExternalInput")
    wv_d = nc.dram_tensor("wv", [P, CI * NH * DH], BF16, kind="ExternalInput")
    wo_d = nc.dram_tensor("wo", [P, NH * OUT_DIM], BF16, kind="ExternalInput")
    xt_d = nc.dram_tensor("xt", [IN_DIM, n], BF16, kind="ExternalInput")
    mk_d = nc.dram_tensor("maskt", [mm, mm], BF16, kind="ExternalInput")
    out_d = nc.dram_tensor("part", [n, OUT_DIM], F32, kind="ExternalOutput")

    xt_v = xt_d.rearrange("(c p) n -> c p n", p=P)
    mk_v = mk_d.rearrange("(j p) i -> p j i", p=P)
    out_v = out_d.rearrange("(t p) o -> t p o", p=P)

    with tile.TileContext(nc) as tc, ExitStack() as ctx:
        const = ctx.enter_context(tc.tile_pool(name="const", bufs=1))

        # Resident inputs (W first: they gate the first matmuls)
        wq = const.tile([P, CI, NH * DH], BF16, tag="wq")
        wk = const.tile([P, CI, NH * DH], BF16, tag="wk")
        wv = const.tile([P, CI, NH * DH], BF16, tag="wv")
        wo = const.tile([P, NH, OUT_DIM], BF16, tag="wo")
        for t, d_ in ((wq, wq_d), (wk, wk_d), (wv, wv_d), (wo, wo_d)):
            nc.sync.dma_start(t[:], d_.rearrange("p (a b) -> p a b", a=t.shape[1]))
        xt = [const.tile([P, n], BF16, tag=f"xt{c}", name=f"xt{c}") for c in range(CI)]
        for c in range(CI):
            nc.sync.dma_start(xt[c][:], xt_v[c])
        ones = const.tile([P, P], BF16, tag="ones")
        nc.vector.memset(ones[:], 1.0)

        # Resident intermediates
        qt = [const.tile([P, n], BF16, tag=f"qt{h}", name=f"qt{h}") for h in range(NH)]
        kt = [const.tile([P, n], BF16, tag=f"kt{h}", name=f"kt{h}") for h in range(NH)]
        vb = const.tile([P, JC, NH * DH], BF16, tag="vb")      # [j, jc, (h d)]
        ot = [const.tile([P, n], BF16, tag=f"ot{h}", name=f"ot{h}") for h in range(NH)]

        # ---- Phase 1: projections ----
        # Q^T, K^T per head: accumulate W[c,h].T @ x^T[c] over c.
        with tc.tile_pool(name="pq", bufs=4, space="PSUM") as pq:
            for h in range(NH):
                for w_sb, dst in ((wq, qt[h]), (wk, kt[h])):
                    for g0 in range(0, IG, 4):
                        gg = range(g0, min(g0 + 4, IG))
                        ps = [pq.tile([P, FD], F32, tag="pq", name="psqk") for _ in gg]
                        for c in range(CI):
                            for gi, g in enumerate(gg):
                                nc.tensor.matmul(
                                    ps[gi][:],
                                    w_sb[:, c, h * DH:(h + 1) * DH],
                                    xt[c][:, g * FD:(g + 1) * FD],
                                    start=(c == 0), stop=(c == CI - 1),
                                )
                        for gi, g in enumerate(gg):
                            nc.vector.tensor_copy(dst[:, g * FD:(g + 1) * FD], ps[gi][:])
            # V (both heads) in natural [seq, d] layout: x^T[c] as weights.
            for t in range(JC):
                ps = pq.tile([P, NH * DH], F32, tag="pv")
                for c in range(CI):
                    nc.tensor.matmul(
                        ps[:], xt[c][:, t * P:(t + 1) * P], wv[:, c, :],
                        start=(c == 0), stop=(c == CI - 1),
                    )
                nc.vector.tensor_copy(vb[:, t, :], ps[:])

        # ---- Phase 2: attention per head ----
        # j-chunks processed in pairs: one [P, 2*FD] exp and one mask multiply
        # per pair halves the ScalarE/VectorE per-op overhead.
        with (
            tc.tile_pool(name="pst", bufs=2, space="PSUM") as pst,
            tc.tile_pool(name="po", bufs=2, space="PSUM") as po,
            tc.tile_pool(name="pd", bufs=2, space="PSUM") as pd,
            tc.tile_pool(name="att", bufs=6) as att,
            tc.tile_pool(name="mkp", bufs=6) as mkp,
        ):
            for h in range(NH):
                for g in range(IG):
                    gs = g * FD
                    oacc = po.tile([P, FD], F32, tag="po")   # [d, i] accum
                    dacc = pd.tile([P, FD], F32, tag="pd")   # bcast denom accum
                    for jp in range(JC // 2):
                        j0 = 2 * jp
                        st2 = pst.tile([P, 2, FD], F32, tag="st")
                        for u in range(2):
                            nc.tensor.matmul(
                                st2[:, u, :],
                                kt[h][:, (j0 + u) * P:(j0 + u + 1) * P],
                                qt[h][:, gs:gs + FD],
                                start=True, stop=True,
                            )
                        pt2 = att.tile([P, 2, FD], BF16, tag="pt")
                        nc.scalar.activation(pt2[:], st2[:], AF.Exp, scale=SCALE)
                        if j0 + 1 < MJ and g < MG:
                            mt2 = mkp.tile([P, 2, FD], BF16, tag="mt")
                            nc.sync.dma_start(
                                mt2[:], mk_v[:, j0:j0 + 2, gs:gs + FD])
                            nc.vector.tensor_mul(out=pt2[:], in0=pt2[:], in1=mt2[:])
                        for u in range(2):
                            nc.tensor.matmul(
                                oacc[:], vb[:, j0 + u, h * DH:(h + 1) * DH],
                                pt2[:, u, :],
                                start=(j0 + u == 0), stop=(j0 + u == JC - 1),
                            )
                            nc.tensor.matmul(
                                dacc[:], ones[:], pt2[:, u, :],
                                start=(j0 + u == 0), stop=(j0 + u == JC - 1),
                            )
                    # 1/den on ScalarE: exp(-ln(den)); both functions live in
                    # one activation-table set, so no table switching.
                    lg = att.tile([P, FD], F32, tag="lg")
                    nc.scalar.activation(lg[:], dacc[:], AF.Ln)
                    rec = att.tile([P, FD], F32, tag="rec")
                    nc.scalar.activation(rec[:], lg[:], AF.Exp, scale=-1.0)
                    nc.vector.tensor_mul(
                        out=ot[h][:, gs:gs + FD], in0=oacc[:], in1=rec[:],
                    )

        # ---- Phase 3: output projection (partial over this core's heads) ----
        with (
            tc.tile_pool(name="pop", bufs=2, space="PSUM") as pop,
            tc.tile_pool(name="osp", bufs=3) as osp,
        ):
            for t in range(JC):
                pso = pop.tile([P, OUT_DIM], F32, tag="pop")
                for h in range(NH):
                    for nf in range(OUT_DIM // FD):
                        nc.tensor.matmul(
                            pso[:, nf * FD:(nf + 1) * FD],
                            ot[h][:, t * P:(t + 1) * P],
                            wo[:, h, nf * FD:(nf + 1) * FD],
                            start=(h == 0), stop=(h == NH - 1),
                        )
                ob = osp.tile([P, OUT_DIM], F32, tag="ob")
                nc.vector.tensor_copy(ob[:], pso[:])
                nc.sync.dma_start(out_v[t], ob[:])

    nc.compile()
    return nc


def make_core_inputs(x, W_qkv, W_out, mask, n=N_FULL, mm=MM_FULL):
    """Host-side shard prep: per-core input dicts (bf16, pre-transposed).

    W slices are delivered in the on-chip layout ([128, c*h*d] with the
    IN_DIM chunk index between partition and column) so the DMA is dense.
    """
    bf = ml_dtypes.bfloat16
    B = x.shape[0]
    CI = IN_DIM // P
    xt_b = [np.ascontiguousarray(x[b].T).astype(bf) for b in range(B)]
    maskt = np.ascontiguousarray(mask[0, 0, :mm, :mm].T).astype(bf)

    def wlayout(w):  # [IN_DIM, NH*DH] -> [P, CI*NH*DH]
        return np.ascontiguousarray(
            w.reshape(CI, P, NH * DH).transpose(1, 0, 2).reshape(P, -1)
        ).astype(bf)

    cores_per_b = N_CORES // B
    in_maps = []
    for core in range(N_CORES):
        b = core // cores_per_b
        h0 = NH * (core % cores_per_b)
        qs, ks, vs = (W_qkv[:, o + h0 * DH: o + (h0 + NH) * DH]
                      for o in (0, OUT_DIM, 2 * OUT_DIM))
        wo_slice = W_out[h0 * DH:(h0 + NH) * DH, :]  # [NH*DH, OUT_DIM]
        wo_l = np.ascontiguousarray(
            wo_slice.reshape(NH, P, OUT_DIM).transpose(1, 0, 2).reshape(P, -1)
        ).astype(bf)
        in_maps.append({
            "xt": xt_b[b],
            "wq": wlayout(qs),
            "wk": wlayout(ks),
            "wv": wlayout(vs),
            "wo": wo_l,
            "maskt": maskt,
        })
    return in_maps


_NC_CACHE = {}


def _get_nc(n=N_FULL, mm=MM_FULL):
    key = (n, mm)
    if key not in _NC_CACHE:
        _NC_CACHE[key] = build_nc(n, mm)
    return _NC_CACHE[key]


def run(x, W_qkv, W_out, b_out, mask, trace=False, **trace_kwargs):
    nc = _get_nc()
    in_maps = make_core_inputs(x, W_qkv, W_out, mask)
    res = run_bass_kernel_spmd(
        nc, in_maps, list(range(N_CORES)), trace=trace, **trace_kwargs
    )
    B = x.shape[0]
    cores_per_b = N_CORES // B
    out = np.zeros((B, N_FULL, OUT_DIM), np.float32)
    for core in range(N_CORES):
        out[core // cores_per_b] += res.results[core]["part"]
    out += np.asarray(b_out, np.float32)
    return out, res


def kernel(x, W_qkv, W_out, b_out, mask, max_mask=MM_FULL, **_ignored):
    x = np.asarray(x, np.float32)
    W_qkv = np.asarray(W_qkv, np.float32)
    W_out = np.asarray(W_out, np.float32)
    b_out = np.asarray(b_out, np.float32)
    mask = np.asarray(mask)
    out, _ = run(x, W_qkv, W_out, b_out, mask)
    return out


# revision 9
# speedup vs baseline: 1.1961x; 1.1961x over previous
"""Bass/Trainium2 kernel for nn_Attention_369367188096 (sparse_attention).

Reference computation (B=2, N=4096, IN_DIM=1024, DIM=1024, HEADS=8, d=128):
    qkv = x @ W_qkv ; split into q,k,v per head
    dots = (q @ k^T) * DIM**-0.5 ; masked on top-left [2048,2048] block
    attn = softmax(dots) ; out = attn @ v ; out @ W_out + b_out

Sharding across 8 NeuronCores: core i handles batch b=i//4 and heads
(2*(i%4), 2*(i%4)+1).  Each core computes a partial output
x[b]-rows x DIM using its two heads' slice of W_out (row-sharded);
the host sums 4 partials per batch and adds b_out.

All matmul operands are bf16 (PE runs bf16 at 1 cycle/row vs 4x for fp32);
accumulation is fp32 in PSUM.  Softmax uses no max-subtraction: scores are
|s| <~ 1.5 after the 1/32 scale, so exp is numerically safe, and masking is
an exact 0/1 multiply after exp (identical to exp(-inf)=0).

Device dataflow (all layouts chosen so matmuls only ever stream, never
transpose): Q^T,K^T = W.T @ x^T with W chunks as PE weights; V natural via
x^T chunks as weights; S^T = K Q^T per (j-chunk, i-group of 512); exp on
ScalarE (scale folded in), 0/1 mask multiply on VectorE; out^T accumulates
V.T @ exp(S^T); the softmax denominator rides a ones-weights matmul whose
output is already broadcast across partitions (chunk pairs pre-summed on
VectorE off the masked region to halve those PE streams); 1/den via VectorE
reciprocal; out^T slices are exactly the lhsT the output projection needs.
"""

import os
import sys

for _p in ("/opt/trn_rl_repo", "/root/.axon_site/_ro/trn_rl_repo"):
    if os.path.isdir(_p) and _p not in sys.path:
        sys.path.insert(0, _p)

from contextlib import ExitStack

import ml_dtypes
import numpy as np

import concourse.bass as bass
import concourse.bacc as bacc
import concourse.mybir as mybir
import concourse.tile as tile
from concourse.bass_utils import run_bass_kernel_spmd

BF16 = mybir.dt.bfloat16
F32 = mybir.dt.float32
P = 128          # partitions
IN_DIM = 1024    # model in dim
OUT_DIM = 1024   # model out dim
DH = 128         # head dim
NH = 2           # heads per core
FD = 512         # matmul moving free dim
N_FULL = 4096    # sequence length
MM_FULL = 2048   # masked block size
SCALE = 1024 ** -0.5
N_CORES = 8


def build_nc(n=N_FULL, mm=MM_FULL):
    """Build the per-core Bass program (SPMD: same program, per-core data)."""
    CI = IN_DIM // P          # 8 input-dim chunks
    JC = n // P               # key chunks (32)
    IG = n // FD              # query groups of 512 (8)
    MJ = mm // P              # masked key chunks (16)
    MG = mm // FD             # masked query groups (4)
    assert MJ % 2 == 0 and JC % 2 == 0
    AF = mybir.ActivationFunctionType

    nc = bacc.Bacc("TRN2", target_bir_lowering=False, debug=False)
    # W tensors arrive host-prelayouted with 128 partitions contiguous so the
    # DMAs are dense and fast (they gate the first matmul).
    wq_d = nc.dram_tensor("wq", [P, CI * NH * DH], BF16, kind="ExternalInput")
    wk_d = nc.dram_tensor("wk", [P, CI * NH * DH], BF16, kind="ExternalInput")
    wv_d = nc.dram_tensor("wv", [P, CI * NH * DH], BF16, kind="ExternalInput")
    wo_d = nc.dram_tensor("wo", [P, NH * OUT_DIM], BF16, kind="ExternalInput")
    xt_d = nc.dram_tensor("xt", [IN_DIM, n], BF16, kind="ExternalInput")
    mk_d = nc.dram_tensor("maskt", [mm, mm], BF16, kind="ExternalInput")
    out_d = nc.dram_tensor("part", [n, OUT_DIM], F32, kind="ExternalOutput")

    xt_v = xt_d.rearrange("(c p) n -> c p n", p=P)
    mk_v = mk_d.rearrange("(j p) i -> p j i", p=P)
    out_v = out_d.rearrange("(t p) o -> t p o", p=P)

    with tile.TileContext(nc) as tc, ExitStack() as ctx:
        const = ctx.enter_context(tc.tile_pool(name="const", bufs=1))

        # Resident inputs (W first: they gate the first matmuls)
        wq = const.tile([P, CI, NH * DH], BF16, tag="wq")
        wk = const.tile([P, CI, NH * DH], BF16, tag="wk")
        wv = const.tile([P, CI, NH * DH], BF16, tag="wv")
        wo = const.tile([P, NH, OUT_DIM], BF16, tag="wo")
        for t, d_ in ((wq, wq_d), (wk, wk_d), (wv, wv_d), (wo, wo_d)):
            nc.sync.dma_start(t[:], d_.rearrange("p (a b) -> p a b", a=t.shape[1]))
        xt = [const.tile([P, n], BF16, tag=f"xt{c}", name=f"xt{c}") for c in range(CI)]
        for c in range(CI):
            nc.sync.dma_start(xt[c][:], xt_v[c])
        ones = const.tile([P, P], BF16, tag="ones")
        nc.vector.memset(ones[:], 1.0)

        # Resident intermediates
        qt = [const.tile([P, n], BF16, tag=f"qt{h}", name=f"qt{h}") for h in range(NH)]
        kt = [const.tile([P, n], BF16, tag=f"kt{h}", name=f"kt{h}") for h in range(NH)]
        vb = const.tile([P, JC, NH * DH], BF16, tag="vb")      # [j, jc, (h d)]
        ot = [const.tile([P, n], BF16, tag=f"ot{h}", name=f"ot{h}") for h in range(NH)]

        # ---- Phase 1: projections ----
        # Q^T, K^T per head: accumulate W[c,h].T @ x^T[c] over c.
        with tc.tile_pool(name="pq", bufs=4, space="PSUM") as pq:
            for h in range(NH):
                for w_sb, dst in ((wq, qt[h]), (wk, kt[h])):
                    for g0 in range(0, IG, 4):
                        gg = range(g0, min(g0 + 4, IG))
                        ps = [pq.tile([P, FD], F32, tag="pq", name="psqk") for _ in gg]
                        for c in range(CI):
                            for gi, g in enumerate(gg):
                                nc.tensor.matmul(
                                    ps[gi][:],
                                    w_sb[:, c, h * DH:(h + 1) * DH],
                                    xt[c][:, g * FD:(g + 1) * FD],
                                    start=(c == 0), stop=(c == CI - 1),
                                )
                        for gi, g in enumerate(gg):
                            nc.any.tensor_copy(dst[:, g * FD:(g + 1) * FD], ps[gi][:])
            # V (both heads) in natural [seq, d] layout: x^T[c] as weights.
            for t in range(JC):
                ps = pq.tile([P, NH * DH], F32, tag="pv")
                for c in range(CI):
                    nc.tensor.matmul(
                        ps[:], xt[c][:, t * P:(t + 1) * P], wv[:, c, :],
                        start=(c == 0), stop=(c == CI - 1),
                    )
                nc.any.tensor_copy(vb[:, t, :], ps[:])

        # ---- Phase 2: attention per head ----
        # j-chunks processed in pairs: one [P, 2*FD] exp and one mask multiply
        # per pair halves the ScalarE/VectorE per-op overhead.
        with (
            tc.tile_pool(name="pst", bufs=2, space="PSUM") as pst,
            tc.tile_pool(name="po", bufs=2, space="PSUM") as po,
            tc.tile_pool(name="pd", bufs=2, space="PSUM") as pd,
            tc.tile_pool(name="att", bufs=6) as att,
            tc.tile_pool(name="mkp", bufs=6) as mkp,
        ):
            for h in range(NH):
                for g in range(IG):
                    gs = g * FD
                    oacc = po.tile([P, FD], F32, tag="po")   # [d, i] accum
                    dacc = pd.tile([P, FD], F32, tag="pd")   # bcast denom accum
                    for jp in range(JC // 2):
                        j0 = 2 * jp
                        st2 = pst.tile([P, 2, FD], F32, tag="st")
                        for u in range(2):
                            nc.tensor.matmul(
                                st2[:, u, :],
                                kt[h][:, (j0 + u) * P:(j0 + u + 1) * P],
                                qt[h][:, gs:gs + FD],
                                start=True, stop=True,
                            )
                        pt2 = att.tile([P, 2, FD], BF16, tag="pt")
                        nc.scalar.activation(pt2[:], st2[:], AF.Exp, scale=SCALE)
                        masked = j0 + 1 < MJ and g < MG
                        if masked:
                            mt2 = mkp.tile([P, 2, FD], BF16, tag="mt")
                            nc.sync.dma_start(
                                mt2[:], mk_v[:, j0:j0 + 2, gs:gs + FD])
                            nc.vector.tensor_mul(out=pt2[:], in0=pt2[:], in1=mt2[:])
                        for u in range(2):
                            nc.tensor.matmul(
                                oacc[:], vb[:, j0 + u, h * DH:(h + 1) * DH],
                                pt2[:, u, :],
                                start=(j0 + u == 0), stop=(j0 + u == JC - 1),
                            )
                        # Denominator: a ones-weights matmul leaves the row sum
                        # already broadcast across partitions.  The [1,FD]-out
                        # stream costs a full FD cycles, so off the masked
                        # region the two chunks are pre-summed on VectorE
                        # (idle there) to halve the PE den streams.
                        if masked:
                            for u in range(2):
                                nc.tensor.matmul(
                                    dacc[:], ones[:], pt2[:, u, :],
                                    start=(j0 + u == 0), stop=(j0 + u == JC - 1),
                                )
                        else:
                            dsum = att.tile([P, FD], BF16, tag="ds")
                            nc.vector.tensor_add(
                                out=dsum[:], in0=pt2[:, 0, :], in1=pt2[:, 1, :])
                            nc.tensor.matmul(
                                dacc[:], ones[:], dsum[:],
                                start=(j0 == 0), stop=(j0 + 1 == JC - 1),
                            )
                    rec = att.tile([P, FD], F32, tag="rec")
                    nc.vector.reciprocal(rec[:], dacc[:])
                    nc.vector.tensor_mul(
                        out=ot[h][:, gs:gs + FD], in0=oacc[:], in1=rec[:],
                    )

        # ---- Phase 3: output projection (partial over this core's heads) ----
        with (
            tc.tile_pool(name="pop", bufs=2, space="PSUM") as pop,
            tc.tile_pool(name="osp", bufs=3) as osp,
        ):
            for t in range(JC):
                pso = pop.tile([P, OUT_DIM], F32, tag="pop")
                for h in range(NH):
                    for nf in range(OUT_DIM // FD):
                        nc.tensor.matmul(
                            pso[:, nf * FD:(nf + 1) * FD],
                            ot[h][:, t * P:(t + 1) * P],
                            wo[:, h, nf * FD:(nf + 1) * FD],
                            start=(h == 0), stop=(h == NH - 1),
                        )
                ob = osp.tile([P, OUT_DIM], F32, tag="ob")
                nc.any.tensor_copy(ob[:], pso[:])
                nc.sync.dma_start(out_v[t], ob[:])

    nc.compile()
    return nc


def make_core_inputs(x, W_qkv, W_out, mask, n=N_FULL, mm=MM_FULL):
    """Host-side shard prep: per-core input dicts (bf16, pre-transposed).

    W slices are delivered in the on-chip layout ([128, c*h*d] with the
    IN_DIM chunk index between partition and column) so the DMA is dense.
    """
    bf = ml_dtypes.bfloat16
    B = x.shape[0]
    CI = IN_DIM // P
    xt_b = [np.ascontiguousarray(x[b].T).astype(bf) for b in range(B)]
    maskt = np.ascontiguousarray(mask[0, 0, :mm, :mm].T).astype(bf)

    def wlayout(w):  # [IN_DIM, NH*DH] -> [P, CI*NH*DH]
        return np.ascontiguousarray(
            w.reshape(CI, P, NH * DH).transpose(1, 0, 2).reshape(P, -1)
        ).astype(bf)

    cores_per_b = N_CORES // B
    in_maps = []
    for core in range(N_CORES):
        b = core // cores_per_b
        h0 = NH * (core % cores_per_b)
        qs, ks, vs = (W_qkv[:, o + h0 * DH: o + (h0 + NH) * DH]
                      for o in (0, OUT_DIM, 2 * OUT_DIM))
        wo_slice = W_out[h0 * DH:(h0 + NH) * DH, :]  # [NH*DH, OUT_DIM]
        wo_l = np.ascontiguousarray(
            wo_slice.reshape(NH, P, OUT_DIM).transpose(1, 0, 2).reshape(P, -1)
        ).astype(bf)
        in_maps.append({
            "xt": xt_b[b],
            "wq": wlayout(qs),
            "wk": wlayout(ks),
            "wv": wlayout(vs),
            "wo": wo_l,
            "maskt": maskt,
        })
    return in_maps


_NC_CACHE = {}


def _get_nc(n=N_FULL, mm=MM_FULL):
    key = (n, mm)
    if key not in _NC_CACHE:
        _NC_CACHE[key] = build_nc(n, mm)
    return _NC_CACHE[key]


def run(x, W_qkv, W_out, b_out, mask, trace=False, **trace_kwargs):
    nc = _get_nc()
    in_maps = make_core_inputs(x, W_qkv, W_out, mask)
    res = run_bass_kernel_spmd(
        nc, in_maps, list(range(N_CORES)), trace=trace, **trace_kwargs
    )
    B = x.shape[0]
    cores_per_b = N_CORES // B
    out = np.zeros((B, N_FULL, OUT_DIM), np.float32)
    for core in range(N_CORES):
        out[core // cores_per_b] += res.results[core]["part"]
    out += np.asarray(b_out, np.float32)
    return out, res


def kernel(x, W_qkv, W_out, b_out, mask, max_mask=MM_FULL, **_ignored):
    x = np.asarray(x, np.float32)
    W_qkv = np.asarray(W_qkv, np.float32)
    W_out = np.asarray(W_out, np.float32)
    b_out = np.asarray(b_out, np.float32)
    mask = np.asarray(mask)
    out, _ = run(x, W_qkv, W_out, b_out, mask)
    return out


# revision 10
# speedup vs baseline: 1.1984x; 1.0020x over previous
"""Bass/Trainium2 kernel for nn_Attention_369367188096 (sparse_attention).

Reference computation (B=2, N=4096, IN_DIM=1024, DIM=1024, HEADS=8, d=128):
    qkv = x @ W_qkv ; split into q,k,v per head
    dots = (q @ k^T) * DIM**-0.5 ; masked on top-left [2048,2048] block
    attn = softmax(dots) ; out = attn @ v ; out @ W_out + b_out

Sharding across 8 NeuronCores: core i handles batch b=i//4 and heads
(2*(i%4), 2*(i%4)+1).  Each core computes a partial output
x[b]-rows x DIM using its two heads' slice of W_out (row-sharded);
the host sums 4 partials per batch and adds b_out.

All matmul operands are bf16 (PE runs bf16 at 1 cycle/row vs 4x for fp32);
accumulation is fp32 in PSUM.  Softmax uses no max-subtraction: scores are
|s| <~ 1.5 after the 1/32 scale, so exp is numerically safe, and masking is
an exact 0/1 multiply after exp (identical to exp(-inf)=0).

Device dataflow (all layouts chosen so matmuls only ever stream, never
transpose): Q^T,K^T = W.T @ x^T with W chunks as PE weights; V natural via
x^T chunks as weights; S^T = K Q^T per (j-chunk, i-group of 512); exp on
ScalarE (scale folded in), 0/1 mask multiply on VectorE; out^T accumulates
V.T @ exp(S^T); the softmax denominator rides a ones-weights matmul whose
output is already broadcast across partitions (chunk pairs pre-summed on
VectorE off the masked region to halve those PE streams); 1/den via VectorE
reciprocal; out^T slices are exactly the lhsT the output projection needs.
"""

import os
import sys

for _p in ("/opt/trn_rl_repo", "/root/.axon_site/_ro/trn_rl_repo"):
    if os.path.isdir(_p) and _p not in sys.path:
        sys.path.insert(0, _p)

from contextlib import ExitStack

import ml_dtypes
import numpy as np

import concourse.bass as bass
import concourse.bacc as bacc
import concourse.mybir as mybir
import concourse.tile as tile
from concourse.bass_utils import run_bass_kernel_spmd

BF16 = mybir.dt.bfloat16
F32 = mybir.dt.float32
P = 128          # partitions
IN_DIM = 1024    # model in dim
OUT_DIM = 1024   # model out dim
DH = 128         # head dim
NH = 2           # heads per core
FD = 512         # matmul moving free dim
N_FULL = 4096    # sequence length
MM_FULL = 2048   # masked block size
SCALE = 1024 ** -0.5
N_CORES = 8


def build_nc(n=N_FULL, mm=MM_FULL):
    """Build the per-core Bass program (SPMD: same program, per-core data)."""
    CI = IN_DIM // P          # 8 input-dim chunks
    JC = n // P               # key chunks (32)
    IG = n // FD              # query groups of 512 (8)
    MJ = mm // P              # masked key chunks (16)
    MG = mm // FD             # masked query groups (4)
    assert MJ % 2 == 0 and JC % 2 == 0
    AF = mybir.ActivationFunctionType

    nc = bacc.Bacc("TRN2", target_bir_lowering=False, debug=False)
    # W tensors arrive host-prelayouted with 128 partitions contiguous so the
    # DMAs are dense and fast (they gate the first matmul).
    wq_d = nc.dram_tensor("wq", [P, CI * NH * DH], BF16, kind="ExternalInput")
    wk_d = nc.dram_tensor("wk", [P, CI * NH * DH], BF16, kind="ExternalInput")
    wv_d = nc.dram_tensor("wv", [P, CI * NH * DH], BF16, kind="ExternalInput")
    wo_d = nc.dram_tensor("wo", [P, NH * OUT_DIM], BF16, kind="ExternalInput")
    xt_d = nc.dram_tensor("xt", [IN_DIM, n], BF16, kind="ExternalInput")
    mk_d = nc.dram_tensor("maskt", [mm, mm], BF16, kind="ExternalInput")
    out_d = nc.dram_tensor("part", [n, OUT_DIM], F32, kind="ExternalOutput")

    xt_v = xt_d.rearrange("(c p) n -> c p n", p=P)
    mk_v = mk_d.rearrange("(j p) i -> p j i", p=P)
    out_v = out_d.rearrange("(t p) o -> t p o", p=P)

    with tile.TileContext(nc) as tc, ExitStack() as ctx:
        const = ctx.enter_context(tc.tile_pool(name="const", bufs=1))

        # Resident inputs (W first: they gate the first matmuls)
        wq = const.tile([P, CI, NH * DH], BF16, tag="wq")
        wk = const.tile([P, CI, NH * DH], BF16, tag="wk")
        wv = const.tile([P, CI, NH * DH], BF16, tag="wv")
        wo = const.tile([P, NH, OUT_DIM], BF16, tag="wo")
        for t, d_ in ((wq, wq_d), (wk, wk_d), (wv, wv_d), (wo, wo_d)):
            nc.sync.dma_start(t[:], d_.rearrange("p (a b) -> p a b", a=t.shape[1]))
        xt = [const.tile([P, n], BF16, tag=f"xt{c}", name=f"xt{c}") for c in range(CI)]
        for c in range(CI):
            nc.sync.dma_start(xt[c][:], xt_v[c])
        ones = const.tile([P, P], BF16, tag="ones")
        nc.vector.memset(ones[:], 1.0)

        # Resident intermediates
        qt = [const.tile([P, n], BF16, tag=f"qt{h}", name=f"qt{h}") for h in range(NH)]
        kt = [const.tile([P, n], BF16, tag=f"kt{h}", name=f"kt{h}") for h in range(NH)]
        vb = const.tile([P, JC, NH * DH], BF16, tag="vb")      # [j, jc, (h d)]
        ot = [const.tile([P, n], BF16, tag=f"ot{h}", name=f"ot{h}") for h in range(NH)]

        # ---- Phase 1: projections ----
        # Q^T, K^T per head: accumulate W[c,h].T @ x^T[c] over c.
        with tc.tile_pool(name="pq", bufs=4, space="PSUM") as pq:
            for h in range(NH):
                for w_sb, dst in ((wq, qt[h]), (wk, kt[h])):
                    for g0 in range(0, IG, 4):
                        gg = range(g0, min(g0 + 4, IG))
                        ps = [pq.tile([P, FD], F32, tag="pq", name="psqk") for _ in gg]
                        for c in range(CI):
                            for gi, g in enumerate(gg):
                                nc.tensor.matmul(
                                    ps[gi][:],
                                    w_sb[:, c, h * DH:(h + 1) * DH],
                                    xt[c][:, g * FD:(g + 1) * FD],
                                    start=(c == 0), stop=(c == CI - 1),
                                )
                        for gi, g in enumerate(gg):
                            nc.any.tensor_copy(dst[:, g * FD:(g + 1) * FD], ps[gi][:])
            # V (both heads) in natural [seq, d] layout: x^T[c] as weights.
            for t in range(JC):
                ps = pq.tile([P, NH * DH], F32, tag="pv")
                for c in range(CI):
                    nc.tensor.matmul(
                        ps[:], xt[c][:, t * P:(t + 1) * P], wv[:, c, :],
                        start=(c == 0), stop=(c == CI - 1),
                    )
                nc.any.tensor_copy(vb[:, t, :], ps[:])

        # ---- Phase 2: attention per head ----
        # j-chunks processed in pairs: one [P, 2*FD] exp and one mask multiply
        # per pair halves the ScalarE/VectorE per-op overhead.
        with (
            tc.tile_pool(name="pst", bufs=2, space="PSUM") as pst,
            tc.tile_pool(name="po", bufs=2, space="PSUM") as po,
            tc.tile_pool(name="pd", bufs=2, space="PSUM") as pd,
            tc.tile_pool(name="att", bufs=6) as att,
            tc.tile_pool(name="mkp", bufs=6) as mkp,
        ):
            # The reciprocal+normalize of i-group g is emitted a few pairs
            # into i-group g+1: VectorE executes in order, and a 3.4us
            # RECIPROCAL at the head of its queue blocks the next group's
            # mask multiplies (which gate PV matmuls -> PE stalls).
            pending = None

            def finalize(pend):
                p_oacc, p_dacc, p_h, p_gs = pend
                rec = att.tile([P, FD], F32, tag="rec", name="rec")
                nc.vector.reciprocal(rec[:], p_dacc[:])
                nc.vector.tensor_mul(
                    out=ot[p_h][:, p_gs:p_gs + FD], in0=p_oacc[:], in1=rec[:],
                )

            for h in range(NH):
                for g in range(IG):
                    gs = g * FD
                    oacc = po.tile([P, FD], F32, tag="po")   # [d, i] accum
                    dacc = pd.tile([P, FD], F32, tag="pd")   # bcast denom accum
                    for jp in range(JC // 2):
                        j0 = 2 * jp
                        st2 = pst.tile([P, 2, FD], F32, tag="st")
                        for u in range(2):
                            nc.tensor.matmul(
                                st2[:, u, :],
                                kt[h][:, (j0 + u) * P:(j0 + u + 1) * P],
                                qt[h][:, gs:gs + FD],
                                start=True, stop=True,
                            )
                        masked = j0 + 1 < MJ and g < MG
                        pt2 = att.tile([P, 2, FD], BF16, tag="pt")
                        mt2 = None
                        if masked:
                            mt2 = mkp.tile([P, 2, FD], BF16, tag="mt")
                            nc.sync.dma_start(
                                mt2[:], mk_v[:, j0:j0 + 2, gs:gs + FD])
                        for u in range(2):
                            # per-chunk exp so PV(u=0) doesn't wait on chunk 1
                            nc.scalar.activation(
                                pt2[:, u, :], st2[:, u, :], AF.Exp, scale=SCALE)
                            if masked:
                                nc.vector.tensor_mul(
                                    out=pt2[:, u, :], in0=pt2[:, u, :],
                                    in1=mt2[:, u, :])
                            nc.tensor.matmul(
                                oacc[:], vb[:, j0 + u, h * DH:(h + 1) * DH],
                                pt2[:, u, :],
                                start=(j0 + u == 0), stop=(j0 + u == JC - 1),
                            )
                        # Denominator: a ones-weights matmul leaves the row sum
                        # already broadcast across partitions.  The [1,FD]-out
                        # stream costs a full FD cycles, so off the masked
                        # region the two chunks are pre-summed on VectorE
                        # (idle there) to halve the PE den streams.
                        if masked:
                            for u in range(2):
                                nc.tensor.matmul(
                                    dacc[:], ones[:], pt2[:, u, :],
                                    start=(j0 + u == 0), stop=(j0 + u == JC - 1),
                                )
                        else:
                            dsum = att.tile([P, FD], BF16, tag="ds")
                            nc.vector.tensor_add(
                                out=dsum[:], in0=pt2[:, 0, :], in1=pt2[:, 1, :])
                            nc.tensor.matmul(
                                dacc[:], ones[:], dsum[:],
                                start=(j0 == 0), stop=(j0 + 1 == JC - 1),
                            )
                        if jp == 3 and pending is not None:
                            finalize(pending)
                            pending = None
                    pending = (oacc, dacc, h, gs)
            finalize(pending)

        # ---- Phase 3: output projection (partial over this core's heads) ----
        with (
            tc.tile_pool(name="pop", bufs=2, space="PSUM") as pop,
            tc.tile_pool(name="osp", bufs=3) as osp,
        ):
            for t in range(JC):
                pso = pop.tile([P, OUT_DIM], F32, tag="pop")
                for h in range(NH):
                    for nf in range(OUT_DIM // FD):
                        nc.tensor.matmul(
                            pso[:, nf * FD:(nf + 1) * FD],
                            ot[h][:, t * P:(t + 1) * P],
                            wo[:, h, nf * FD:(nf + 1) * FD],
                            start=(h == 0), stop=(h == NH - 1),
                        )
                ob = osp.tile([P, OUT_DIM], F32, tag="ob")
                nc.any.tensor_copy(ob[:], pso[:])
                nc.sync.dma_start(out_v[t], ob[:])

    nc.compile()
    return nc


def make_core_inputs(x, W_qkv, W_out, mask, n=N_FULL, mm=MM_FULL):
    """Host-side shard prep: per-core input dicts (bf16, pre-transposed).

    W slices are delivered in the on-chip layout ([128, c*h*d] with the
    IN_DIM chunk index between partition and column) so the DMA is dense.
    """
    bf = ml_dtypes.bfloat16
    B = x.shape[0]
    CI = IN_DIM // P
    xt_b = [np.ascontiguousarray(x[b].T).astype(bf) for b in range(B)]
    maskt = np.ascontiguousarray(mask[0, 0, :mm, :mm].T).astype(bf)

    def wlayout(w):  # [IN_DIM, NH*DH] -> [P, CI*NH*DH]
        return np.ascontiguousarray(
            w.reshape(CI, P, NH * DH).transpose(1, 0, 2).reshape(P, -1)
        ).astype(bf)

    cores_per_b = N_CORES // B
    in_maps = []
    for core in range(N_CORES):
        b = core // cores_per_b
        h0 = NH * (core % cores_per_b)
        qs, ks, vs = (W_qkv[:, o + h0 * DH: o + (h0 + NH) * DH]
                      for o in (0, OUT_DIM, 2 * OUT_DIM))
        wo_slice = W_out[h0 * DH:(h0 + NH) * DH, :]  # [NH*DH, OUT_DIM]
        wo_l = np.ascontiguousarray(
            wo_slice.reshape(NH, P, OUT_DIM).transpose(1, 0, 2).reshape(P, -1)
        ).astype(bf)
        in_maps.append({
            "xt": xt_b[b],
            "wq": wlayout(qs),
            "wk": wlayout(ks),
            "wv": wlayout(vs),
            "wo": wo_l,
            "maskt": maskt,
        })
    return in_maps


_NC_CACHE = {}


def _get_nc(n=N_FULL, mm=MM_FULL):
    key = (n, mm)
    if key not in _NC_CACHE:
        _NC_CACHE[key] = build_nc(n, mm)
    return _NC_CACHE[key]


def run(x, W_qkv, W_out, b_out, mask, trace=False, **trace_kwargs):
    nc = _get_nc()
    in_maps = make_core_inputs(x, W_qkv, W_out, mask)
    res = run_bass_kernel_spmd(
        nc, in_maps, list(range(N_CORES)), trace=trace, **trace_kwargs
    )
    B = x.shape[0]
    cores_per_b = N_CORES // B
    out = np.zeros((B, N_FULL, OUT_DIM), np.float32)
    for core in range(N_CORES):
        out[core // cores_per_b] += res.results[core]["part"]
    out += np.asarray(b_out, np.float32)
    return out, res


def kernel(x, W_qkv, W_out, b_out, mask, max_mask=MM_FULL, **_ignored):
    x = np.asarray(x, np.float32)
    W_qkv = np.asarray(W_qkv, np.float32)
    W_out = np.asarray(W_out, np.float32)
    b_out = np.asarray(b_out, np.float32)
    mask = np.asarray(mask)
    out, _ = run(x, W_qkv, W_out, b_out, mask)
    return out
